# revision 1
# baseline (speedup 1.0000x reference)
"""Trainium2 Bass kernel for nn_Attention_46067819217077 (sparse_attention).

Computation (reference):
  x (64,2,32,32) -> flatten (b=64, n=2, dim=1024)
  q/k/v = BN1d_n( x @ W{q,k,v}.T )          (inner = 2048 = 2 heads x 1024)
  linear attention per (b, head):  out = (s @ v_hat) * D_inv  with
      s[n,m] = q_hat[n] . k_hat[m],  D[n] = s[n,0]+s[n,1]   (seq len n = 2)
  out = merge_heads @ Wo.T + bo ; BN2d over (b, H, W)

Strategy: 8-way tensor-parallel on the inner (head-feature) axis.
Each core owns a 256-wide chunk of Wq/Wk/Wv rows and of Wo columns.
BatchNorm statistics, q.k dot products and row sums are all *linear* in
per-chunk partial sums, so one small AllGather (+local sum) provides
everything needed to form normalized attention weights after the fact.
The Wo matmul is applied to raw per-chunk V (Y = V @ WoC.T) *before* the
attention weights are known (it overlaps the AllGather); the attention
combine then happens on Y with per-row scalars:
  partial = u_diag*Y + u_off*Y_swap + z (x) wo_colsum + bo/8
A final AllReduce sums the 8 partials; BN2 is computed redundantly on
every core.

Row layout everywhere: r = n*64 + b  (channel-major, 128 rows).
"""

import numpy as np

NC = 8
B, N, H, W = 64, 2, 32, 32
DIM = H * W                # 1024
INNER = DIM * 2            # 2048
DPC = INNER // NC          # 256 per-core chunk
EPS = 1e-5

_PROG_CACHE = {}

# Matmul operand dtype: "f32" (exact, 4 cycles/row) or "f32r" (fast fp32,
# 1 cycle/row at N>=256, reduced precision). Switch via MM_DT before first use.
MM_DT = "f32"
NO_CC = False  # debug: replace collectives with local DMAs (wrong results, timing only)


def _build_program(mm_dt=None, reps=1):
    import concourse.bass as bass
    import concourse.mybir as mybir
    import concourse.tile as tile
    from concourse import bacc

    f32 = mybir.dt.float32
    fmm = mybir.dt.float32r if (mm_dt or MM_DT) == "f32r" else mybir.dt.float32
    A = mybir.AluOpType
    AF = mybir.ActivationFunctionType
    AX = mybir.AxisListType

    no_cc = NO_CC
    nc = bacc.Bacc(None, target_bir_lowering=False, debug=False, num_devices=NC)

    # ---- I/O ----
    d_xt = nc.dram_tensor("xt", [128, 8, 128], fmm, kind="ExternalInput")
    d_wqk = nc.dram_tensor("wqk", [128, 8, 512], fmm, kind="ExternalInput")
    d_wv = nc.dram_tensor("wv", [128, 8, 256], fmm, kind="ExternalInput")
    d_wo = nc.dram_tensor("wo", [128, 2, 1024], fmm, kind="ExternalInput")
    d_wos = nc.dram_tensor("wos", [1024], f32, kind="ExternalInput")
    d_bo8 = nc.dram_tensor("bo8", [1024], f32, kind="ExternalInput")
    d_hm = nc.dram_tensor("hm", [128, 2], f32, kind="ExternalInput")
    d_mn = nc.dram_tensor("mn", [128, 2], f32, kind="ExternalInput")
    d_m2 = nc.dram_tensor("m2", [2, 128], f32, kind="ExternalInput")
    d_m2o = nc.dram_tensor("m2o", [2, 128], f32, kind="ExternalInput")
    d_gb = nc.dram_tensor("gb", [2, 8], f32, kind="ExternalInput")
    d_out = nc.dram_tensor("out", [128, 1024], f32, kind="ExternalOutput")

    cc1_in = nc.dram_tensor("cc1_in", [128, 16], f32)
    cc1_out = nc.dram_tensor("cc1_out", [NC * 128, 16], f32, addr_space="Shared")
    cc2_in = nc.dram_tensor("cc2_in", [128, 1024], f32)
    cc2_out = nc.dram_tensor("cc2_out", [128, 1024], f32, addr_space="Shared")

    def bcast(ap, p=128):
        return bass.AP(tensor=ap.tensor, offset=ap.offset, ap=[[0, p]] + list(ap.ap))

    groups = [list(range(NC))]

    with tile.TileContext(nc) as tc:
        with (
            tc.tile_pool(name="const", bufs=1) as cst_pool,
            tc.tile_pool(name="work", bufs=1) as wk,
            tc.tile_pool(name="psum", bufs=1, space="PSUM") as ps,
            tc.tile_pool(name="psvt", bufs=2, space="PSUM") as psvt,
        ):
            for _rep in range(reps):
                # ---- constant loads (chunked for DMA/compute overlap) ----
                t_xt = [cst_pool.tile([128, 128], fmm, name=f"xt{c}", tag=f"xt{c}") for c in range(8)]
                t_wqk = [cst_pool.tile([128, 512], fmm, name=f"wqk{c}", tag=f"wqk{c}") for c in range(8)]
                t_wv = [cst_pool.tile([128, 256], fmm, name=f"wv{c}", tag=f"wv{c}") for c in range(8)]
                t_wo = [cst_pool.tile([128, 1024], fmm, name=f"wo{k}", tag=f"wo{k}") for k in range(2)]
                t_wos = cst_pool.tile([128, 1024], f32, name="wos", tag="wos")
                t_bo8 = cst_pool.tile([128, 1024], f32, name="bo8", tag="bo8")
                t_hm = cst_pool.tile([128, 2], f32, name="hm", tag="hm")
                t_mn = cst_pool.tile([128, 2], f32, name="mn", tag="mn")
                t_m2 = cst_pool.tile([2, 128], f32, name="m2", tag="m2")
                t_m2o = cst_pool.tile([2, 128], f32, name="m2o", tag="m2o")
                t_gb = cst_pool.tile([2, 8], f32, name="gb", tag="gb")

                for c in range(8):
                    nc.sync.dma_start(out=t_xt[c], in_=d_xt[:, c, :])
                    nc.sync.dma_start(out=t_wqk[c], in_=d_wqk[:, c, :])
                    nc.sync.dma_start(out=t_wv[c], in_=d_wv[:, c, :])
                for k in range(2):
                    nc.sync.dma_start(out=t_wo[k], in_=d_wo[:, k, :])
                nc.sync.dma_start(out=t_hm, in_=d_hm[:])
                nc.sync.dma_start(out=t_mn, in_=d_mn[:])
                nc.sync.dma_start(out=t_m2, in_=d_m2[:])
                nc.sync.dma_start(out=t_m2o, in_=d_m2o[:])
                nc.sync.dma_start(out=t_gb, in_=d_gb[:])
                nc.sync.dma_start(out=t_wos, in_=bcast(d_wos[:]))
                nc.sync.dma_start(out=t_bo8, in_=bcast(d_bo8[:]))

                # ---- phase 1: projections ----
                # QK: out[r, j2] += xt_c.T @ wqk_c    (contract d)
                qk_ps = ps.tile([128, 512], f32, name="qk", tag="qk")
                for c in range(8):
                    nc.tensor.matmul(qk_ps, lhsT=t_xt[c], rhs=t_wqk[c],
                                     start=(c == 0), stop=(c == 7))
                # V^T tiles: out[j, r] += wv_c.T @ xt_c
                vt_ps = [psvt.tile([128, 128], f32, name="vt", tag="vt") for _ in range(2)]
                for half in range(2):
                    for c in range(8):
                        nc.tensor.matmul(vt_ps[half], lhsT=t_wv[c][:, half * 128:(half + 1) * 128],
                                         rhs=t_xt[c], start=(c == 0), stop=(c == 7))
                # ---- V^T to SBUF for later Y matmuls ----
                vts = wk.tile([128, 2, 128], fmm, name="vts", tag="vts")
                for half in range(2):
                    nc.scalar.copy(out=vts[:, half, :], in_=vt_ps[half])

                # ---- phase 2: payload (per-chunk partial sums) ----
                q_ap = qk_ps[:, 0:256]
                k_ap = qk_ps[:, 256:512]
                # K to SBUF (walrus: only one PSUM input per DVE op), plus
                # a partition-swapped copy
                ksb = wk.tile([128, 256], f32, name="ksb", tag="ksb")
                nc.scalar.copy(out=ksb, in_=k_ap)
                ksw = wk.tile([128, 256], f32, name="ksw", tag="ksw")
                nc.vector.tensor_copy(out=ksw[0:64, :], in_=ksb[64:128, :])
                nc.vector.tensor_copy(out=ksw[64:128, :], in_=ksb[0:64, :])

                tmp4 = wk.tile([128, 4], f32, name="tmp4", tag="tmp4")
                prod1 = wk.tile([128, 256], f32, name="prod1", tag="prod1")
                prod2 = wk.tile([128, 256], f32, name="prod2", tag="prod2")
                nc.vector.tensor_tensor(out=prod1, in0=q_ap, in1=ksb, op=A.mult)
                nc.vector.tensor_reduce(out=tmp4[:, 0:1], in_=prod1, axis=AX.X, op=A.add)
                nc.vector.tensor_tensor(out=prod2, in0=q_ap, in1=ksw, op=A.mult)
                nc.vector.tensor_reduce(out=tmp4[:, 1:2], in_=prod2, axis=AX.X, op=A.add)
                nc.vector.tensor_reduce(out=tmp4[:, 2:4],
                                        in_=qk_ps[:].rearrange("p (t j) -> p t j", t=2),
                                        axis=AX.X, op=A.add)

                pay = wk.tile([128, 16], f32, name="pay", tag="pay")
                nc.vector.memset(pay, 0.0)
                nc.vector.tensor_scalar(out=pay[:, 0:4], in0=tmp4, scalar1=t_hm[:, 0:1],
                                        scalar2=None, op0=A.mult)
                nc.vector.tensor_scalar(out=pay[:, 4:8], in0=tmp4, scalar1=t_hm[:, 1:2],
                                        scalar2=None, op0=A.mult)
                sq1 = wk.tile([128, 256], f32, name="sq1", tag="sq1")
                sq2 = wk.tile([128, 256], f32, name="sq2", tag="sq2")
                nc.scalar.activation(out=sq1, in_=q_ap, func=AF.Square, accum_out=pay[:, 8:9])
                nc.scalar.activation(out=sq2, in_=k_ap, func=AF.Square, accum_out=pay[:, 9:10])
                vsq = [wk.tile([128, 128], f32, name=f"vsq{i}", tag=f"vsq{i}") for i in range(2)]
                v2ab = [wk.tile([128, 2], f32, name=f"v2ab{i}", tag=f"v2ab{i}") for i in range(2)]
                for half in range(2):
                    nc.scalar.activation(out=vsq[half], in_=vt_ps[half], func=AF.Square)
                    nc.vector.tensor_reduce(out=v2ab[half],
                                            in_=vsq[half][:].rearrange("p (t r) -> p t r", t=2),
                                            axis=AX.X, op=A.add)
                nc.vector.tensor_tensor(out=pay[:, 11:13], in0=v2ab[0], in1=v2ab[1], op=A.add)
                vsab = [wk.tile([128, 2], f32, name=f"vsab{i}", tag=f"vsab{i}") for i in range(2)]
                for half in range(2):
                    nc.vector.tensor_reduce(out=vsab[half],
                                            in_=vt_ps[half][:].rearrange("p (t r) -> p t r", t=2),
                                            axis=AX.X, op=A.add)
                nc.vector.tensor_tensor(out=pay[:, 13:15], in0=vsab[0], in1=vsab[1], op=A.add)

                # ---- collective 1: AllGather payload, local sum ----
                nc.sync.dma_start(out=cc1_in[:], in_=pay)
                if no_cc is True:
                    nc.gpsimd.dma_start(out=cc1_out[0:128, :], in_=cc1_in[:])
                else:
                    nc.gpsimd.collective_compute(
                        "AllGather", A.bypass, replica_groups=groups,
                        ins=[cc1_in[:]], outs=[cc1_out[:]])
                gat = wk.tile([128, 8, 16], f32, name="gat", tag="gat")
                nc.sync.dma_start(out=gat, in_=cc1_out[:].rearrange("(c p) f -> p c f", p=128))
                S = wk.tile([128, 16], f32, name="S", tag="S")
                nc.vector.tensor_reduce(out=S, in_=gat[:].rearrange("p c f -> p f c"),
                                        axis=AX.X, op=A.add)

                # ---- Y matmuls (overlap the AllGather): Y = V @ WoC.T ----
                y_ps = ps.tile([128, 1024], f32, name="y", tag="y")
                for k in range(2):
                    for nn in range(2):
                        nc.tensor.matmul(y_ps[:, nn * 512:(nn + 1) * 512],
                                         lhsT=vts[:, k, :], rhs=t_wo[k][:, nn * 512:(nn + 1) * 512],
                                         start=(k == 0), stop=(k == 1))
                # row-swapped copy of Y (hidden in the AllGather window)
                ysw_sb = wk.tile([128, 1024], f32, name="ysw_sb", tag="ysw_sb")
                nc.vector.tensor_copy(out=ysw_sb[0:64, :], in_=y_ps[64:128, :])
                nc.vector.tensor_copy(out=ysw_sb[64:128, :], in_=y_ps[0:64, :])

                # ---- post-gather: head-slot select ----
                S4 = wk.tile([128, 4], f32, name="S4", tag="S4")
                th = wk.tile([128, 4], f32, name="th", tag="th")
                nc.vector.tensor_scalar(out=th, in0=S[:, 0:4], scalar1=t_hm[:, 0:1],
                                        scalar2=None, op0=A.mult)
                nc.vector.scalar_tensor_tensor(out=S4, in0=S[:, 4:8], scalar=t_hm[:, 1:2],
                                               in1=th, op0=A.mult, op1=A.add)
                # kr of the opposite row (partition swap)
                kr2 = wk.tile([128, 2], f32, name="kr2", tag="kr2")
                nc.vector.tensor_copy(out=kr2[:, 0:1], in_=S4[:, 3:4])
                nc.vector.tensor_copy(out=kr2[0:64, 1:2], in_=S4[64:128, 3:4])
                nc.vector.tensor_copy(out=kr2[64:128, 1:2], in_=S4[0:64, 3:4])

                # ---- global BN1 stats ----
                rhs4 = wk.tile([128, 4], f32, name="rhs4", tag="rhs4")
                nc.vector.tensor_tensor(out=rhs4[:, 0:2], in0=S[:, 2:4], in1=S[:, 6:8], op=A.add)
                nc.vector.tensor_copy(out=rhs4[:, 2:4], in_=S[:, 8:10])
                ones1 = wk.tile([128, 1], f32, name="ones1", tag="ones1")
                nc.vector.memset(ones1, 1.0)
                st_ps = ps.tile([128, 16], f32, name="st", tag="qk")
                # [Sq Sk SSq SSk] per channel
                nc.tensor.matmul(out=st_ps[0:2, 0:4], lhsT=t_mn, rhs=rhs4, start=True, stop=True)
                # SSv and Sv per channel from per-j payload partials
                nc.tensor.matmul(out=st_ps[0:2, 4:5], lhsT=S[:, 11:13], rhs=ones1,
                                 start=True, stop=True)
                nc.tensor.matmul(out=st_ps[0:2, 5:6], lhsT=S[:, 13:15], rhs=ones1,
                                 start=True, stop=True)
                sts = wk.tile([2, 6], f32, name="sts", tag="sts")   # [Sq Sk Sv SSq SSk SSv]
                nc.vector.tensor_copy(out=sts[:, 0:2], in_=st_ps[0:2, 0:2])
                nc.vector.tensor_copy(out=sts[:, 2:3], in_=st_ps[0:2, 5:6])
                nc.vector.tensor_copy(out=sts[:, 3:5], in_=st_ps[0:2, 2:4])
                nc.vector.tensor_copy(out=sts[:, 5:6], in_=st_ps[0:2, 4:5])

                cst = wk.tile([2, 32], f32, name="cst", tag="cst")
                eps_t = wk.tile([2, 1], f32, name="eps_t", tag="eps_t")
                nc.vector.memset(eps_t, EPS)
                inv_n1 = 1.0 / float(B * INNER)
                nc.vector.tensor_scalar(out=cst[:, 0:3], in0=sts[:, 0:3], scalar1=inv_n1,
                                        scalar2=None, op0=A.mult)          # means
                nc.vector.tensor_scalar(out=cst[:, 3:6], in0=sts[:, 3:6], scalar1=inv_n1,
                                        scalar2=None, op0=A.mult)          # E[x^2]
                nc.vector.tensor_tensor(out=cst[:, 6:9], in0=cst[:, 0:3], in1=cst[:, 0:3], op=A.mult)
                nc.vector.tensor_tensor(out=cst[:, 9:12], in0=cst[:, 3:6], in1=cst[:, 6:9], op=A.subtract)
                nc.scalar.activation(out=cst[:, 12:15], in_=cst[:, 9:12], func=AF.Sqrt,
                                     bias=eps_t, scale=1.0)
                nc.vector.reciprocal(out=cst[:, 15:18], in_=cst[:, 12:15])
                nc.vector.tensor_tensor(out=cst[:, 18:21], in0=t_gb[0:2, 0:3], in1=cst[:, 15:18],
                                        op=A.mult)                          # A = g*rstd
                nc.vector.tensor_tensor(out=cst[:, 24:27], in0=cst[:, 18:21], in1=cst[:, 0:3],
                                        op=A.mult)                          # A*mean
                nc.vector.tensor_tensor(out=cst[:, 21:24], in0=t_gb[0:2, 3:6], in1=cst[:, 24:27],
                                        op=A.subtract)                      # C = b - A*mean

                # broadcast per-channel constants to rows: same + opposite channel
                bc_ps = ps.tile([128, 16], f32, name="bc", tag="vr")
                nc.tensor.matmul(out=bc_ps[:, 0:6], lhsT=t_m2, rhs=cst[:, 18:24],
                                 start=True, stop=True)
                nc.tensor.matmul(out=bc_ps[:, 6:12], lhsT=t_m2o, rhs=cst[:, 18:24],
                                 start=True, stop=True)
                bc = wk.tile([128, 12], f32, name="bc_sb", tag="bc_sb")
                nc.scalar.copy(out=bc, in_=bc_ps[:, 0:12])
                # bc cols: 0 Aq 1 Ak 2 Av 3 Cq 4 Ck 5 Cv | 6 Aq' 7 Ak' 8 Av' 9 Cq' 10 Ck' 11 Cv'

                # ---- scores ----
                # CACD[:, 0:8] = [AqAk, AqCk, AqAk', AqCk', CqAk, CqCk, CqAk', CqCk']
                # via one tensor_tensor with broadcast/strided APs over bc
                CACD = wk.tile([128, 8], f32, name="CACD", tag="CACD")
                in0 = bass.AP(tensor=bc.tensor, offset=bc.offset,
                              ap=[list(bc.ap[0]), [3, 2], [0, 4]])       # [Aq x4, Cq x4]
                in1 = bass.AP(tensor=bc.tensor, offset=bc.offset + 1,
                              ap=[list(bc.ap[0]), [0, 2], [3, 4]])       # [Ak Ck Ak' Ck'] x2
                nc.vector.tensor_tensor(out=CACD[:].rearrange("p (a b) -> p a b", a=2),
                                        in0=in0, in1=in1, op=A.mult)
                nc.vector.tensor_scalar(out=CACD[:, 5:6], in0=CACD[:, 5:6], scalar1=float(DIM),
                                        scalar2=None, op0=A.mult)
                nc.vector.tensor_scalar(out=CACD[:, 7:8], in0=CACD[:, 7:8], scalar1=float(DIM),
                                        scalar2=None, op0=A.mult)

                def cacd(k):
                    return bass.AP(tensor=CACD.tensor, offset=CACD.offset + k,
                                   ap=[list(CACD.ap[0]), [2, 2]])
                CA, CB, CC, CD = cacd(0), cacd(1), cacd(4), cacd(5)

                sc = wk.tile([128, 2], f32, name="sc", tag="sc")
                t3 = wk.tile([128, 2], f32, name="t3", tag="t3")
                nc.vector.tensor_tensor(out=sc, in0=CA, in1=S4[:, 0:2], op=A.mult)
                nc.vector.scalar_tensor_tensor(out=sc, in0=CB, scalar=S4[:, 2:3], in1=sc,
                                               op0=A.mult, op1=A.add)
                nc.vector.tensor_tensor(out=t3, in0=CC, in1=kr2, op=A.mult)
                nc.vector.tensor_tensor(out=sc, in0=sc, in1=t3, op=A.add)
                nc.vector.tensor_tensor(out=sc, in0=sc, in1=CD, op=A.add)

                Dcol = wk.tile([128, 1], f32, name="Dcol", tag="Dcol")
                Din = wk.tile([128, 1], f32, name="Din", tag="Din")
                nc.vector.tensor_reduce(out=Dcol, in_=sc, axis=AX.X, op=A.add)
                nc.vector.reciprocal(out=Din, in_=Dcol)
                w2 = wk.tile([128, 2], f32, name="w2", tag="w2")
                nc.vector.tensor_scalar(out=w2, in0=sc, scalar1=Din, scalar2=None, op0=A.mult)
                uz = wk.tile([128, 3], f32, name="uz", tag="uz")   # [u_diag, u_off, z]
                t4 = wk.tile([128, 1], f32, name="t4", tag="t4")
                nc.vector.tensor_scalar(out=uz[:, 0:1], in0=w2[:, 0:1], scalar1=bc[:, 2:3], scalar2=None, op0=A.mult)
                nc.vector.tensor_scalar(out=uz[:, 1:2], in0=w2[:, 1:2], scalar1=bc[:, 8:9], scalar2=None, op0=A.mult)
                nc.vector.tensor_scalar(out=t4, in0=w2[:, 0:1], scalar1=bc[:, 5:6], scalar2=None, op0=A.mult)
                nc.vector.scalar_tensor_tensor(out=uz[:, 2:3], in0=w2[:, 1:2], scalar=bc[:, 11:12],
                                               in1=t4, op0=A.mult, op1=A.add)

                # ---- combine: partial = u_diag*Y + u_off*Ysw + z*wo_sum + bo/8 ----
                Rt = wk.tile([128, 1024], f32, name="Rt", tag="Rt")
                nc.vector.scalar_tensor_tensor(out=Rt, in0=t_wos, scalar=uz[:, 2:3], in1=t_bo8,
                                               op0=A.mult, op1=A.add)
                t2 = wk.tile([128, 1024], f32, name="t2", tag="t2")
                nc.vector.scalar_tensor_tensor(out=t2, in0=ysw_sb, scalar=uz[:, 1:2], in1=Rt,
                                               op0=A.mult, op1=A.add)
                outp = wk.tile([128, 1024], f32, name="outp", tag="outp")
                nc.vector.scalar_tensor_tensor(out=outp, in0=y_ps, scalar=uz[:, 0:1], in1=t2,
                                               op0=A.mult, op1=A.add)

                # ---- collective 2: AllReduce partials ----
                nc.sync.dma_start(out=cc2_in[:], in_=outp)
                if no_cc in (True, "no_ar"):
                    nc.gpsimd.dma_start(out=cc2_out[:], in_=cc2_in[:])
                else:
                    nc.gpsimd.collective_compute(
                        "AllReduce", A.add, replica_groups=groups,
                        ins=[cc2_in[:]], outs=[cc2_out[:]])
                Xt = wk.tile([128, 1024], f32, name="Xt", tag="Xt")
                nc.sync.dma_start(out=Xt[:, 0:512], in_=cc2_out[:, 0:512])
                nc.sync.dma_start(out=Xt[:, 512:1024], in_=cc2_out[:, 512:1024])

                # ---- BN2 (halves overlap the Xt DMA) ----
                r2h = wk.tile([128, 2, 2], f32, name="r2h", tag="r2h")
                scr2 = wk.tile([128, 1024], f32, name="scr2", tag="scr2")
                for hh in range(2):
                    cols = slice(hh * 512, (hh + 1) * 512)
                    nc.vector.tensor_reduce(out=r2h[:, hh, 0:1], in_=Xt[:, cols], axis=AX.X, op=A.add)
                    nc.scalar.activation(out=scr2[:, cols], in_=Xt[:, cols], func=AF.Square,
                                         accum_out=r2h[:, hh, 1:2])
                r2 = wk.tile([128, 2], f32, name="r2", tag="r2")
                nc.vector.tensor_tensor(out=r2, in0=r2h[:, 0, :], in1=r2h[:, 1, :], op=A.add)
                st2_ps = ps.tile([128, 4], f32, name="st2", tag="qk")
                nc.tensor.matmul(out=st2_ps[0:2, 0:2], lhsT=t_mn, rhs=r2, start=True, stop=True)
                cst2 = wk.tile([2, 12], f32, name="cst2", tag="cst2")
                inv_n2 = 1.0 / float(B * DIM)
                nc.vector.tensor_scalar(out=cst2[:, 0:2], in0=st2_ps[0:2, 0:2], scalar1=inv_n2,
                                        scalar2=None, op0=A.mult)           # [mean, E2]
                nc.vector.tensor_tensor(out=cst2[:, 2:3], in0=cst2[:, 0:1], in1=cst2[:, 0:1], op=A.mult)
                nc.vector.tensor_tensor(out=cst2[:, 3:4], in0=cst2[:, 1:2], in1=cst2[:, 2:3], op=A.subtract)
                nc.scalar.activation(out=cst2[:, 4:5], in_=cst2[:, 3:4], func=AF.Sqrt,
                                     bias=eps_t, scale=1.0)
                nc.vector.reciprocal(out=cst2[:, 5:6], in_=cst2[:, 4:5])
                nc.vector.tensor_tensor(out=cst2[:, 6:7], in0=t_gb[0:2, 6:7], in1=cst2[:, 5:6], op=A.mult)  # abn
                nc.vector.tensor_tensor(out=cst2[:, 8:9], in0=cst2[:, 6:7], in1=cst2[:, 0:1], op=A.mult)
                nc.vector.tensor_tensor(out=cst2[:, 7:8], in0=t_gb[0:2, 7:8], in1=cst2[:, 8:9], op=A.subtract)  # cbn
                bc2_ps = ps.tile([128, 4], f32, name="bc2", tag="vr")
                nc.tensor.matmul(out=bc2_ps[:, 0:2], lhsT=t_m2, rhs=cst2[:, 6:8], start=True, stop=True)
                bc2 = wk.tile([128, 2], f32, name="bc2_sb", tag="bc2_sb")
                nc.scalar.copy(out=bc2, in_=bc2_ps[:, 0:2])
                fin = wk.tile([128, 1024], f32, name="fin", tag="fin")
                for hh in range(2):
                    cols = slice(hh * 512, (hh + 1) * 512)
                    nc.vector.tensor_scalar(out=fin[:, cols], in0=Xt[:, cols], scalar1=bc2[:, 0:1],
                                            scalar2=bc2[:, 1:2], op0=A.mult, op1=A.add)
                    nc.sync.dma_start(out=d_out[:, cols], in_=fin[:, cols])

    nc.compile()
    return nc


def _prep_inputs(x, Wq, Wk, Wv, Wo, bo, g_q, b_q, g_k, b_k, g_v, b_v, g_bn, b_bn):
    f = np.float32
    x, Wq, Wk, Wv, Wo, bo = (np.asarray(t, f) for t in (x, Wq, Wk, Wv, Wo, bo))
    g_q, b_q, g_k, b_k, g_v, b_v, g_bn, b_bn = (
        np.asarray(t, f) for t in (g_q, b_q, g_k, b_k, g_v, b_v, g_bn, b_bn))
    x = np.ascontiguousarray(x, f)
    xf = x.reshape(B, N, DIM)
    Xr = np.ascontiguousarray(xf.transpose(1, 0, 2).reshape(N * B, DIM))   # n-major rows
    xt = np.ascontiguousarray(Xr.T.reshape(8, 128, 128).transpose(1, 0, 2))  # [p, c, r]

    mn = np.zeros((128, 2), f)
    mn[0:64, 0] = 1.0
    mn[64:128, 1] = 1.0
    m2 = np.ascontiguousarray(mn.T)            # (2, 128)
    m2o = np.ascontiguousarray(mn[:, ::-1].T)  # opposite channel
    gb = np.stack([g_q, g_k, g_v, b_q, b_k, b_v, g_bn, b_bn], axis=1).astype(f)
    bo8 = (np.asarray(bo, f) / NC).astype(f)

    in_maps = []
    for i in range(NC):
        rows = slice(i * DPC, (i + 1) * DPC)
        head = i // 4
        wqk_c = np.concatenate([Wq[rows], Wk[rows]], axis=0).astype(f)       # (512, 1024)
        wqk = np.ascontiguousarray(wqk_c.T.reshape(8, 128, 512).transpose(1, 0, 2))
        wv_c = np.asarray(Wv[rows], f)                                        # (256, 1024)
        wv = np.ascontiguousarray(wv_c.T.reshape(8, 128, 256).transpose(1, 0, 2))
        WoC = np.asarray(Wo[:, rows], f)                                      # (1024, 256)
        wo = np.ascontiguousarray(WoC.T.reshape(2, 128, 1024).transpose(1, 0, 2))
        wos = np.ascontiguousarray(WoC.sum(1))                                # (1024,)
        hm = np.zeros((128, 2), f)
        hm[:, head] = 1.0
        in_maps.append({
            "xt": xt, "wqk": wqk, "wv": wv, "wo": wo,
            "wos": wos, "bo8": bo8, "hm": hm, "mn": mn, "m2": m2,
            "m2o": m2o, "gb": gb,
        })
    return in_maps


def _postprocess(out128):
    return np.ascontiguousarray(
        out128.reshape(N, B, DIM).transpose(1, 0, 2).reshape(B, N, H, W)
    ).astype(np.float32)


def _get_program(reps=1):
    key = ("nc", MM_DT, reps, NO_CC)
    if key not in _PROG_CACHE:
        _PROG_CACHE[key] = _build_program(MM_DT, reps)
    return _PROG_CACHE[key]


def kernel(**inputs):
    from concourse.bass_utils import run_bass_kernel_spmd
    nc = _get_program()
    in_maps = _prep_inputs(**inputs)
    res = run_bass_kernel_spmd(nc, in_maps, list(range(NC)))
    return _postprocess(res.results[0]["out"])


def run_traced(inputs):
    """Like kernel() but with NTFF tracing; returns (output, BassKernelResults)."""
    from concourse.bass_utils import run_bass_kernel_spmd
    nc = _get_program()
    in_maps = _prep_inputs(**inputs)
    res = run_bass_kernel_spmd(nc, in_maps, list(range(NC)), trace=True)
    return _postprocess(res.results[0]["out"]), res


def run_sim(inputs):
    """Validate in the multi-core simulator; returns output."""
    from concourse.bass_interp import MultiCoreSim
    nc = _get_program()
    in_maps = _prep_inputs(**inputs)
    sim = MultiCoreSim(nc, num_cores=NC, trace=False)
    for i in range(NC):
        for k, v in in_maps[i].items():
            sim.cores[i].tensor(k)[:] = v
    sim.simulate()
    return _postprocess(np.array(sim.cores[0].tensor("out")))



# revision 6
# speedup vs baseline: 3.9945x; 3.9945x over previous
"""Trainium2 Bass kernel for nn_Attention_46067819217077 (sparse_attention).

Computation (reference):
  x (64,2,32,32) -> flatten (b=64, n=2, dim=1024)
  q/k/v = BN1d_n( x @ W{q,k,v}.T )          (inner = 2048 = 2 heads x 1024)
  linear attention per (b, head):  out = (s @ v_hat) * D_inv  with
      s[n,m] = q_hat[n] . k_hat[m],  D[n] = s[n,0]+s[n,1]   (seq len n = 2)
  out = merge_heads @ Wo.T + bo ; BN2d over (b, H, W)

Strategy: 8-way tensor-parallel on the inner (head-feature) axis.
Each core owns a 256-wide chunk of Wq/Wk/Wv rows and of Wo columns.
BatchNorm statistics, q.k dot products and row sums are all *linear* in
per-chunk partial sums, so one small AllGather (+local sum) provides
everything needed to form normalized attention weights after the fact.
The Wo matmul is applied to raw per-chunk V (Y = V @ WoC.T) *before* the
attention weights are known; the attention combine then happens on Y
with per-row scalars:
  partial = u_diag*Y + u_off*Y_swap + z (x) wo_colsum + bo/8
A final AllReduce (bf16) sums the 8 partials; BN2 is computed
redundantly on every core.

Perf structure vs the original version:
  - QK projections in f32r (1 cycle/row), V / Y path in bf16.
  - AllReduce payload in bf16 (halves ring bytes).
  - z (x) wo_colsum + bo/8 via a tiny K=2 outer-product matmul instead
    of 1 MB/rep of broadcast DMAs.
  - Software-pipelined reps in three stages: P1 (weight DMA, matmuls,
    payload, AllGather), P2a (scores, combine, AllReduce), P2b (BN2
    tail, output).  Emission order is P1_{i}, P2a_{i-1}, P2b_{i-2} so
    every engine queue keeps the next rep's front-end work AHEAD of
    AllReduce-gated tail ops, and the steady state is bound by the
    collective fabric (~AG+AR per rep) instead of the serial chain.

Row layout everywhere: r = n*64 + b  (channel-major, 128 rows).
"""

import numpy as np

NC = 8
B, N, H, W = 64, 2, 32, 32
DIM = H * W                # 1024
INNER = DIM * 2            # 2048
DPC = INNER // NC          # 256 per-core chunk
EPS = 1e-5

_PROG_CACHE = {}

# QK matmul operand dtype: "f32r" (fast fp32, 1 cycle/row) or "f32" (exact,
# 4 cycles/row). The V/Wo path is always bf16.
MM_DT = "f32r"
NO_CC = False  # debug: replace collectives with local DMAs (wrong results, timing only)


def _build_program(mm_dt=None, reps=1):
    import concourse.bass as bass
    import concourse.mybir as mybir
    import concourse.tile as tile
    from concourse import bacc

    f32 = mybir.dt.float32
    bf16 = mybir.dt.bfloat16
    fqk = mybir.dt.float32r if (mm_dt or MM_DT) == "f32r" else mybir.dt.float32
    A = mybir.AluOpType
    AF = mybir.ActivationFunctionType
    AX = mybir.AxisListType

    no_cc = NO_CC
    nc = bacc.Bacc(None, target_bir_lowering=False, debug=False, num_devices=NC)

    # ---- I/O ----
    d_xt = nc.dram_tensor("xt", [128, 8, 128], fqk, kind="ExternalInput")
    d_xtb = nc.dram_tensor("xtb", [128, 8, 128], bf16, kind="ExternalInput")
    d_wqk = nc.dram_tensor("wqk", [128, 8, 512], fqk, kind="ExternalInput")
    d_wvb = nc.dram_tensor("wvb", [128, 8, 256], bf16, kind="ExternalInput")
    d_wob = nc.dram_tensor("wob", [128, 2, 1024], bf16, kind="ExternalInput")
    d_wosbo = nc.dram_tensor("wosbo", [2, 1024], bf16, kind="ExternalInput")
    d_eye = nc.dram_tensor("eye", [128, 128], f32, kind="ExternalInput")
    d_hm = nc.dram_tensor("hm", [128, 2], f32, kind="ExternalInput")
    d_mn = nc.dram_tensor("mn", [128, 2], f32, kind="ExternalInput")
    d_m2 = nc.dram_tensor("m2", [2, 128], f32, kind="ExternalInput")
    d_m2o = nc.dram_tensor("m2o", [2, 128], f32, kind="ExternalInput")
    d_gb = nc.dram_tensor("gb", [2, 8], f32, kind="ExternalInput")
    d_out = nc.dram_tensor("out", [128, 1024], f32, kind="ExternalOutput")

    cc1_in = [nc.dram_tensor(f"cc1_in{p}", [128, 16], f32) for p in range(2)]
    cc1_out = [nc.dram_tensor(f"cc1_out{p}", [NC * 128, 16], f32, addr_space="Shared")
               for p in range(2)]
    cc2_in = [nc.dram_tensor(f"cc2_in{p}", [128, 1024], bf16) for p in range(2)]
    cc2_out = [nc.dram_tensor(f"cc2_out{p}", [128, 1024], bf16, addr_space="Shared")
               for p in range(2)]

    groups = [list(range(NC))]

    with tile.TileContext(nc) as tc:
        with (
            tc.tile_pool(name="const", bufs=2) as cst,
            tc.tile_pool(name="cst3", bufs=3) as cst3,
            tc.tile_pool(name="p1", bufs=1) as w1,
            tc.tile_pool(name="x12", bufs=2) as w12,
            tc.tile_pool(name="p2", bufs=1) as w2p,
            tc.tile_pool(name="ps_qk", bufs=1, space="PSUM") as ps_qk,
            tc.tile_pool(name="ps_vt", bufs=1, space="PSUM") as ps_vt,
            tc.tile_pool(name="ps_y", bufs=1, space="PSUM") as ps_y,
            tc.tile_pool(name="ps_sma", bufs=1, space="PSUM") as ps_sma,
            tc.tile_pool(name="ps_smb", bufs=1, space="PSUM") as ps_smb,
            tc.tile_pool(name="ps_rt", bufs=1, space="PSUM") as ps_rt,
        ):

            def phase1(i):
                par = i % 2
                st = {}
                # ---- constant loads ----
                t_xt = [cst.tile([128, 128], fqk, name=f"xt{c}", tag=f"xt{c}") for c in range(8)]
                t_wqk = [cst.tile([128, 512], fqk, name=f"wqk{c}", tag=f"wqk{c}") for c in range(8)]
                t_xtb = [cst.tile([128, 128], bf16, name=f"xtb{c}", tag=f"xtb{c}") for c in range(8)]
                t_wvb = [cst.tile([128, 256], bf16, name=f"wvb{c}", tag=f"wvb{c}") for c in range(8)]
                t_wob = [cst.tile([128, 1024], bf16, name=f"wob{k}", tag=f"wob{k}") for k in range(2)]
                st["wosbo"] = t_wosbo = cst.tile([2, 1024], bf16, name="wosbo", tag="wosbo")
                st["eye"] = t_eye = cst.tile([128, 128], f32, name="eye", tag="eye")
                st["hm"] = t_hm = cst.tile([128, 2], f32, name="hm", tag="hm")
                st["m2o"] = cst.tile([2, 128], f32, name="m2o", tag="m2o")
                # read by both P2a (one rep back) and P2b (two reps back): bufs=3
                st["mn"] = t_mn = cst3.tile([128, 2], f32, name="mn", tag="mn")
                st["m2"] = cst3.tile([2, 128], f32, name="m2", tag="m2")
                st["gb"] = cst3.tile([2, 8], f32, name="gb", tag="gb")

                for c in range(8):
                    nc.sync.dma_start(out=t_xt[c], in_=d_xt[:, c, :])
                    nc.sync.dma_start(out=t_wqk[c], in_=d_wqk[:, c, :])
                for c in range(8):
                    nc.sync.dma_start(out=t_xtb[c], in_=d_xtb[:, c, :])
                    nc.sync.dma_start(out=t_wvb[c], in_=d_wvb[:, c, :])
                for k in range(2):
                    nc.sync.dma_start(out=t_wob[k], in_=d_wob[:, k, :])
                nc.sync.dma_start(out=t_wosbo, in_=d_wosbo[:])
                nc.sync.dma_start(out=t_eye, in_=d_eye[:])
                nc.sync.dma_start(out=t_hm, in_=d_hm[:])
                nc.sync.dma_start(out=t_mn, in_=d_mn[:])
                nc.sync.dma_start(out=st["m2"], in_=d_m2[:])
                nc.sync.dma_start(out=st["m2o"], in_=d_m2o[:])
                nc.sync.dma_start(out=st["gb"], in_=d_gb[:])

                # ---- projections ----
                qk_ps = ps_qk.tile([128, 512], f32, name="qk", tag="qk")
                for c in range(8):
                    nc.tensor.matmul(qk_ps, lhsT=t_xt[c], rhs=t_wqk[c],
                                     start=(c == 0), stop=(c == 7))
                vt_ps = ps_vt.tile([128, 256], f32, name="vt", tag="vt")
                for half in range(2):
                    for c in range(8):
                        nc.tensor.matmul(vt_ps[:, half * 128:(half + 1) * 128],
                                         lhsT=t_wvb[c][:, half * 128:(half + 1) * 128],
                                         rhs=t_xtb[c], start=(c == 0), stop=(c == 7))
                # V^T to SBUF (bf16) for the Y matmuls
                vts = w1.tile([128, 2, 128], bf16, name="vts", tag="vts")
                for half in range(2):
                    nc.scalar.copy(out=vts[:, half, :], in_=vt_ps[:, half * 128:(half + 1) * 128])

                # ---- payload (per-chunk partial sums) ----
                q_ap = qk_ps[:, 0:256]
                k_ap = qk_ps[:, 256:512]
                ksb = w1.tile([128, 256], f32, name="ksb", tag="ksb")
                nc.scalar.copy(out=ksb, in_=k_ap)
                ksw = w1.tile([128, 256], f32, name="ksw", tag="ksw")
                nc.vector.tensor_copy(out=ksw[0:64, :], in_=ksb[64:128, :])
                nc.vector.tensor_copy(out=ksw[64:128, :], in_=ksb[0:64, :])

                tmp4 = w1.tile([128, 4], f32, name="tmp4", tag="tmp4")
                prod1 = w1.tile([128, 256], f32, name="prod1", tag="prod1")
                prod2 = w1.tile([128, 256], f32, name="prod2", tag="prod2")
                nc.vector.tensor_tensor(out=prod1, in0=q_ap, in1=ksb, op=A.mult)
                nc.vector.tensor_reduce(out=tmp4[:, 0:1], in_=prod1, axis=AX.X, op=A.add)
                nc.vector.tensor_tensor(out=prod2, in0=q_ap, in1=ksw, op=A.mult)
                nc.vector.tensor_reduce(out=tmp4[:, 1:2], in_=prod2, axis=AX.X, op=A.add)
                nc.vector.tensor_reduce(out=tmp4[:, 2:4],
                                        in_=qk_ps[:].rearrange("p (t j) -> p t j", t=2),
                                        axis=AX.X, op=A.add)

                pay = w1.tile([128, 16], f32, name="pay", tag="pay")
                nc.vector.memset(pay, 0.0)
                nc.vector.tensor_scalar(out=pay[:, 0:4], in0=tmp4, scalar1=t_hm[:, 0:1],
                                        scalar2=None, op0=A.mult)
                nc.vector.tensor_scalar(out=pay[:, 4:8], in0=tmp4, scalar1=t_hm[:, 1:2],
                                        scalar2=None, op0=A.mult)
                sq1 = w1.tile([128, 256], f32, name="sq1", tag="sq1")
                sq2 = w1.tile([128, 256], f32, name="sq2", tag="sq2")
                nc.scalar.activation(out=sq1, in_=q_ap, func=AF.Square, accum_out=pay[:, 8:9])
                nc.scalar.activation(out=sq2, in_=k_ap, func=AF.Square, accum_out=pay[:, 9:10])
                vsq = [w1.tile([128, 128], f32, name=f"vsq{i}", tag=f"vsq{i}") for i in range(2)]
                v2ab = [w1.tile([128, 2], f32, name=f"v2ab{i}", tag=f"v2ab{i}") for i in range(2)]
                for half in range(2):
                    nc.scalar.activation(out=vsq[half],
                                         in_=vt_ps[:, half * 128:(half + 1) * 128],
                                         func=AF.Square)
                    nc.vector.tensor_reduce(out=v2ab[half],
                                            in_=vsq[half][:].rearrange("p (t r) -> p t r", t=2),
                                            axis=AX.X, op=A.add)
                nc.vector.tensor_tensor(out=pay[:, 11:13], in0=v2ab[0], in1=v2ab[1], op=A.add)
                vsab = [w1.tile([128, 2], f32, name=f"vsab{i}", tag=f"vsab{i}") for i in range(2)]
                for half in range(2):
                    nc.vector.tensor_reduce(out=vsab[half],
                                            in_=vt_ps[:, half * 128:(half + 1) * 128]
                                            .rearrange("p (t r) -> p t r", t=2),
                                            axis=AX.X, op=A.add)
                nc.vector.tensor_tensor(out=pay[:, 13:15], in0=vsab[0], in1=vsab[1], op=A.add)

                # ---- collective 1: AllGather payload ----
                nc.gpsimd.dma_start(out=cc1_in[par][:], in_=pay)
                if no_cc is True:
                    nc.gpsimd.dma_start(out=cc1_out[par][0:128, :], in_=cc1_in[par][:])
                else:
                    nc.gpsimd.collective_compute(
                        "AllGather", A.bypass, replica_groups=groups,
                        ins=[cc1_in[par][:]], outs=[cc1_out[par][:]])

                # ---- Y matmuls: Y = V @ WoC.T ----
                y_ps = ps_y.tile([128, 1024], f32, name="y", tag="y")
                for k in range(2):
                    for nn in range(2):
                        nc.tensor.matmul(y_ps[:, nn * 512:(nn + 1) * 512],
                                         lhsT=vts[:, k, :],
                                         rhs=t_wob[k][:, nn * 512:(nn + 1) * 512],
                                         start=(k == 0), stop=(k == 1))
                # bf16 copies: plain (scalar engine) + row-swapped (vector)
                ysb = w12.tile([128, 1024], bf16, name="ysb", tag="ysb")
                nc.scalar.copy(out=ysb, in_=y_ps)
                ysw = w12.tile([128, 1024], bf16, name="ysw", tag="ysw")
                nc.vector.tensor_copy(out=ysw[0:64, :], in_=y_ps[64:128, :])
                nc.vector.tensor_copy(out=ysw[64:128, :], in_=y_ps[0:64, :])
                st["ysb"], st["ysw"] = ysb, ysw
                return st

            def phase2a(i, st):
                par = i % 2
                t_hm, t_mn, t_m2, t_m2o = st["hm"], st["mn"], st["m2"], st["m2o"]
                t_gb, t_wosbo, t_eye = st["gb"], st["wosbo"], st["eye"]
                ysb, ysw = st["ysb"], st["ysw"]

                # ---- gather readback, local sum ----
                gat = w2p.tile([128, 8, 16], f32, name="gat", tag="gat")
                nc.sync.dma_start(out=gat,
                                  in_=cc1_out[par][:].rearrange("(c p) f -> p c f", p=128))
                S = w2p.tile([128, 16], f32, name="S", tag="S")
                nc.vector.tensor_reduce(out=S, in_=gat[:].rearrange("p c f -> p f c"),
                                        axis=AX.X, op=A.add)

                # ---- head-slot select ----
                S4 = w2p.tile([128, 4], f32, name="S4", tag="S4")
                th = w2p.tile([128, 4], f32, name="th", tag="th")
                nc.vector.tensor_scalar(out=th, in0=S[:, 0:4], scalar1=t_hm[:, 0:1],
                                        scalar2=None, op0=A.mult)
                nc.vector.scalar_tensor_tensor(out=S4, in0=S[:, 4:8], scalar=t_hm[:, 1:2],
                                               in1=th, op0=A.mult, op1=A.add)
                kr2 = w2p.tile([128, 2], f32, name="kr2", tag="kr2")
                nc.vector.tensor_copy(out=kr2[:, 0:1], in_=S4[:, 3:4])
                nc.vector.tensor_copy(out=kr2[0:64, 1:2], in_=S4[64:128, 3:4])
                nc.vector.tensor_copy(out=kr2[64:128, 1:2], in_=S4[0:64, 3:4])

                # ---- global BN1 stats ----
                rhs4 = w2p.tile([128, 4], f32, name="rhs4", tag="rhs4")
                nc.vector.tensor_tensor(out=rhs4[:, 0:2], in0=S[:, 2:4], in1=S[:, 6:8], op=A.add)
                nc.vector.tensor_copy(out=rhs4[:, 2:4], in_=S[:, 8:10])
                ones1 = w2p.tile([128, 1], f32, name="ones1", tag="ones1")
                nc.vector.memset(ones1, 1.0)
                sma = ps_sma.tile([128, 160], f32, name="sma", tag="sma")
                # st region: [Sq Sk SSq SSk] + SSv + Sv per channel
                nc.tensor.matmul(out=sma[0:2, 0:4], lhsT=t_mn, rhs=rhs4, start=True, stop=True)
                nc.tensor.matmul(out=sma[0:2, 4:5], lhsT=S[:, 11:13], rhs=ones1,
                                 start=True, stop=True)
                nc.tensor.matmul(out=sma[0:2, 5:6], lhsT=S[:, 13:15], rhs=ones1,
                                 start=True, stop=True)
                sts = w2p.tile([2, 6], f32, name="sts", tag="sts")  # [Sq Sk Sv SSq SSk SSv]
                nc.vector.tensor_copy(out=sts[:, 0:2], in_=sma[0:2, 0:2])
                nc.vector.tensor_copy(out=sts[:, 2:3], in_=sma[0:2, 5:6])
                nc.vector.tensor_copy(out=sts[:, 3:5], in_=sma[0:2, 2:4])
                nc.vector.tensor_copy(out=sts[:, 5:6], in_=sma[0:2, 4:5])

                cst_t = w2p.tile([2, 32], f32, name="cst", tag="cst")
                eps_t = w2p.tile([2, 1], f32, name="eps_t", tag="eps_t")
                nc.vector.memset(eps_t, EPS)
                inv_n1 = 1.0 / float(B * INNER)
                nc.vector.tensor_scalar(out=cst_t[:, 0:3], in0=sts[:, 0:3], scalar1=inv_n1,
                                        scalar2=None, op0=A.mult)           # means
                nc.vector.tensor_scalar(out=cst_t[:, 3:6], in0=sts[:, 3:6], scalar1=inv_n1,
                                        scalar2=None, op0=A.mult)           # E[x^2]
                nc.vector.tensor_tensor(out=cst_t[:, 6:9], in0=cst_t[:, 0:3], in1=cst_t[:, 0:3],
                                        op=A.mult)
                nc.vector.tensor_tensor(out=cst_t[:, 9:12], in0=cst_t[:, 3:6], in1=cst_t[:, 6:9],
                                        op=A.subtract)
                nc.scalar.activation(out=cst_t[:, 12:15], in_=cst_t[:, 9:12], func=AF.Sqrt,
                                     bias=eps_t, scale=1.0)
                nc.vector.reciprocal(out=cst_t[:, 15:18], in_=cst_t[:, 12:15])
                nc.vector.tensor_tensor(out=cst_t[:, 18:21], in0=t_gb[0:2, 0:3],
                                        in1=cst_t[:, 15:18], op=A.mult)     # A = g*rstd
                nc.vector.tensor_tensor(out=cst_t[:, 24:27], in0=cst_t[:, 18:21],
                                        in1=cst_t[:, 0:3], op=A.mult)       # A*mean
                nc.vector.tensor_tensor(out=cst_t[:, 21:24], in0=t_gb[0:2, 3:6],
                                        in1=cst_t[:, 24:27], op=A.subtract) # C = b - A*mean

                # broadcast per-channel constants to rows: same + opposite channel
                nc.tensor.matmul(out=sma[:, 8:14], lhsT=t_m2, rhs=cst_t[:, 18:24],
                                 start=True, stop=True)
                nc.tensor.matmul(out=sma[:, 14:20], lhsT=t_m2o, rhs=cst_t[:, 18:24],
                                 start=True, stop=True)
                bc = w2p.tile([128, 12], f32, name="bc_sb", tag="bc_sb")
                nc.scalar.copy(out=bc, in_=sma[:, 8:20])
                # bc cols: 0 Aq 1 Ak 2 Av 3 Cq 4 Ck 5 Cv | 6 Aq' 7 Ak' 8 Av' 9 Cq' 10 Ck' 11 Cv'

                # ---- scores ----
                CACD = w2p.tile([128, 8], f32, name="CACD", tag="CACD")
                in0 = bass.AP(tensor=bc.tensor, offset=bc.offset,
                              ap=[list(bc.ap[0]), [3, 2], [0, 4]])       # [Aq x4, Cq x4]
                in1 = bass.AP(tensor=bc.tensor, offset=bc.offset + 1,
                              ap=[list(bc.ap[0]), [0, 2], [3, 4]])       # [Ak Ck Ak' Ck'] x2
                nc.vector.tensor_tensor(out=CACD[:].rearrange("p (a b) -> p a b", a=2),
                                        in0=in0, in1=in1, op=A.mult)
                nc.vector.tensor_scalar(out=CACD[:, 5:6], in0=CACD[:, 5:6], scalar1=float(DIM),
                                        scalar2=None, op0=A.mult)
                nc.vector.tensor_scalar(out=CACD[:, 7:8], in0=CACD[:, 7:8], scalar1=float(DIM),
                                        scalar2=None, op0=A.mult)

                def cacd(k):
                    return bass.AP(tensor=CACD.tensor, offset=CACD.offset + k,
                                   ap=[list(CACD.ap[0]), [2, 2]])
                CA, CB, CC, CD = cacd(0), cacd(1), cacd(4), cacd(5)

                sc = w2p.tile([128, 2], f32, name="sc", tag="sc")
                t3 = w2p.tile([128, 2], f32, name="t3", tag="t3")
                nc.vector.tensor_tensor(out=sc, in0=CA, in1=S4[:, 0:2], op=A.mult)
                nc.vector.scalar_tensor_tensor(out=sc, in0=CB, scalar=S4[:, 2:3], in1=sc,
                                               op0=A.mult, op1=A.add)
                nc.vector.tensor_tensor(out=t3, in0=CC, in1=kr2, op=A.mult)
                nc.vector.tensor_tensor(out=sc, in0=sc, in1=t3, op=A.add)
                nc.vector.tensor_tensor(out=sc, in0=sc, in1=CD, op=A.add)

                Dcol = w2p.tile([128, 1], f32, name="Dcol", tag="Dcol")
                Din = w2p.tile([128, 1], f32, name="Din", tag="Din")
                nc.vector.tensor_reduce(out=Dcol, in_=sc, axis=AX.X, op=A.add)
                nc.vector.reciprocal(out=Din, in_=Dcol)
                w2 = w2p.tile([128, 2], f32, name="w2", tag="w2")
                nc.vector.tensor_scalar(out=w2, in0=sc, scalar1=Din, scalar2=None, op0=A.mult)
                uz = w2p.tile([128, 4], f32, name="uz", tag="uz")  # [u_diag, u_off, z, 1]
                t4 = w2p.tile([128, 1], f32, name="t4", tag="t4")
                nc.vector.tensor_scalar(out=uz[:, 0:1], in0=w2[:, 0:1], scalar1=bc[:, 2:3],
                                        scalar2=None, op0=A.mult)
                nc.vector.tensor_scalar(out=uz[:, 1:2], in0=w2[:, 1:2], scalar1=bc[:, 8:9],
                                        scalar2=None, op0=A.mult)
                nc.vector.tensor_scalar(out=t4, in0=w2[:, 0:1], scalar1=bc[:, 5:6],
                                        scalar2=None, op0=A.mult)
                nc.vector.scalar_tensor_tensor(out=uz[:, 2:3], in0=w2[:, 1:2],
                                               scalar=bc[:, 11:12], in1=t4,
                                               op0=A.mult, op1=A.add)

                # ---- Rt = z (x) wos + 1 (x) bo/8 via K=2 outer-product matmul ----
                nc.vector.memset(uz[:, 3:4], 1.0)
                nc.tensor.transpose(out=sma[0:2, 32:160], in_=uz[:, 2:4], identity=t_eye)
                ztb = w2p.tile([2, 128], bf16, name="ztb", tag="ztb")
                nc.vector.tensor_copy(out=ztb, in_=sma[0:2, 32:160])
                rt_ps = ps_rt.tile([128, 1024], f32, name="rt", tag="rt")
                for nn in range(2):
                    nc.tensor.matmul(out=rt_ps[:, nn * 512:(nn + 1) * 512], lhsT=ztb,
                                     rhs=t_wosbo[:, nn * 512:(nn + 1) * 512],
                                     start=True, stop=True)

                # ---- combine: partial = u_diag*Y + u_off*Ysw + Rt ----
                t2 = w2p.tile([128, 1024], f32, name="t2", tag="t2")
                nc.vector.scalar_tensor_tensor(out=t2, in0=ysw, scalar=uz[:, 1:2], in1=rt_ps,
                                               op0=A.mult, op1=A.add)
                outp = w2p.tile([128, 1024], bf16, name="outp", tag="outp")
                nc.vector.scalar_tensor_tensor(out=outp, in0=ysb, scalar=uz[:, 0:1], in1=t2,
                                               op0=A.mult, op1=A.add)

                # ---- collective 2: AllReduce partials (bf16) ----
                nc.sync.dma_start(out=cc2_in[par][:], in_=outp)
                if no_cc in (True, "no_ar"):
                    nc.gpsimd.dma_start(out=cc2_out[par][:], in_=cc2_in[par][:])
                else:
                    nc.gpsimd.collective_compute(
                        "AllReduce", A.add, replica_groups=groups,
                        ins=[cc2_in[par][:]], outs=[cc2_out[par][:]])

            def phase2b(i, st):
                par = i % 2
                t_mn, t_m2, t_gb = st["mn"], st["m2"], st["gb"]

                Xt = w2p.tile([128, 1024], bf16, name="Xt", tag="Xt")
                nc.scalar.dma_start(out=Xt, in_=cc2_out[par][:])

                # ---- BN2 ----
                r2h = w2p.tile([128, 2, 2], f32, name="r2h", tag="r2h")
                scr2 = w2p.tile([128, 1024], f32, name="scr2", tag="scr2")
                for hh in range(2):
                    cols = slice(hh * 512, (hh + 1) * 512)
                    nc.vector.tensor_reduce(out=r2h[:, hh, 0:1], in_=Xt[:, cols],
                                            axis=AX.X, op=A.add)
                    nc.scalar.activation(out=scr2[:, cols], in_=Xt[:, cols], func=AF.Square,
                                         accum_out=r2h[:, hh, 1:2])
                r2 = w2p.tile([128, 2], f32, name="r2", tag="r2")
                nc.vector.tensor_tensor(out=r2, in0=r2h[:, 0, :], in1=r2h[:, 1, :], op=A.add)
                smb = ps_smb.tile([128, 8], f32, name="smb", tag="smb")
                nc.tensor.matmul(out=smb[0:2, 0:2], lhsT=t_mn, rhs=r2, start=True, stop=True)
                cst2 = w2p.tile([2, 12], f32, name="cst2", tag="cst2")
                eps2_t = w2p.tile([2, 1], f32, name="eps2_t", tag="eps2_t")
                nc.vector.memset(eps2_t, EPS)
                inv_n2 = 1.0 / float(B * DIM)
                nc.vector.tensor_scalar(out=cst2[:, 0:2], in0=smb[0:2, 0:2], scalar1=inv_n2,
                                        scalar2=None, op0=A.mult)           # [mean, E2]
                nc.vector.tensor_tensor(out=cst2[:, 2:3], in0=cst2[:, 0:1], in1=cst2[:, 0:1],
                                        op=A.mult)
                nc.vector.tensor_tensor(out=cst2[:, 3:4], in0=cst2[:, 1:2], in1=cst2[:, 2:3],
                                        op=A.subtract)
                nc.scalar.activation(out=cst2[:, 4:5], in_=cst2[:, 3:4], func=AF.Sqrt,
                                     bias=eps2_t, scale=1.0)
                nc.vector.reciprocal(out=cst2[:, 5:6], in_=cst2[:, 4:5])
                nc.vector.tensor_tensor(out=cst2[:, 6:7], in0=t_gb[0:2, 6:7], in1=cst2[:, 5:6],
                                        op=A.mult)                          # abn
                nc.vector.tensor_tensor(out=cst2[:, 8:9], in0=cst2[:, 6:7], in1=cst2[:, 0:1],
                                        op=A.mult)
                nc.vector.tensor_tensor(out=cst2[:, 7:8], in0=t_gb[0:2, 7:8], in1=cst2[:, 8:9],
                                        op=A.subtract)                      # cbn
                nc.tensor.matmul(out=smb[:, 2:4], lhsT=t_m2, rhs=cst2[:, 6:8],
                                 start=True, stop=True)
                bc2 = w2p.tile([128, 2], f32, name="bc2_sb", tag="bc2_sb")
                nc.scalar.copy(out=bc2, in_=smb[:, 2:4])
                fin = w2p.tile([128, 1024], f32, name="fin", tag="fin")
                nc.vector.tensor_scalar(out=fin, in0=Xt, scalar1=bc2[:, 0:1],
                                        scalar2=bc2[:, 1:2], op0=A.mult, op1=A.add)
                nc.scalar.dma_start(out=d_out[:], in_=fin)

            # ---- software-pipelined emission: P1_i | P2a_{i-1} | P2b_{i-2} ----
            P1 = [None] * reps
            P1[0] = phase1(0)
            if reps > 1:
                P1[1] = phase1(1)
            phase2a(0, P1[0])
            for i in range(2, reps):
                P1[i] = phase1(i)
                phase2a(i - 1, P1[i - 1])
                phase2b(i - 2, P1[i - 2])
            if reps > 1:
                phase2a(reps - 1, P1[reps - 1])
                phase2b(reps - 2, P1[reps - 2])
            phase2b(reps - 1, P1[reps - 1])

    nc.compile()
    return nc


def _prep_inputs(x, Wq, Wk, Wv, Wo, bo, g_q, b_q, g_k, b_k, g_v, b_v, g_bn, b_bn):
    import ml_dtypes
    f = np.float32
    bf = ml_dtypes.bfloat16
    x, Wq, Wk, Wv, Wo, bo = (np.asarray(t, f) for t in (x, Wq, Wk, Wv, Wo, bo))
    g_q, b_q, g_k, b_k, g_v, b_v, g_bn, b_bn = (
        np.asarray(t, f) for t in (g_q, b_q, g_k, b_k, g_v, b_v, g_bn, b_bn))
    x = np.ascontiguousarray(x, f)
    xf = x.reshape(B, N, DIM)
    Xr = np.ascontiguousarray(xf.transpose(1, 0, 2).reshape(N * B, DIM))   # n-major rows
    xt = np.ascontiguousarray(Xr.T.reshape(8, 128, 128).transpose(1, 0, 2))  # [p, c, r]
    xtb = xt.astype(bf)

    mn = np.zeros((128, 2), f)
    mn[0:64, 0] = 1.0
    mn[64:128, 1] = 1.0
    m2 = np.ascontiguousarray(mn.T)            # (2, 128)
    m2o = np.ascontiguousarray(mn[:, ::-1].T)  # opposite channel
    gb = np.stack([g_q, g_k, g_v, b_q, b_k, b_v, g_bn, b_bn], axis=1).astype(f)
    eye = np.eye(128, dtype=f)

    in_maps = []
    for i in range(NC):
        rows = slice(i * DPC, (i + 1) * DPC)
        head = i // 4
        wqk_c = np.concatenate([Wq[rows], Wk[rows]], axis=0).astype(f)       # (512, 1024)
        wqk = np.ascontiguousarray(wqk_c.T.reshape(8, 128, 512).transpose(1, 0, 2))
        wv_c = np.asarray(Wv[rows], f)                                        # (256, 1024)
        wvb = np.ascontiguousarray(wv_c.T.reshape(8, 128, 256).transpose(1, 0, 2)).astype(bf)
        WoC = np.asarray(Wo[:, rows], f)                                      # (1024, 256)
        wob = np.ascontiguousarray(WoC.T.reshape(2, 128, 1024).transpose(1, 0, 2)).astype(bf)
        wosbo = np.stack([WoC.sum(1), bo / NC], axis=0).astype(bf)            # (2, 1024)
        hm = np.zeros((128, 2), f)
        hm[:, head] = 1.0
        in_maps.append({
            "xt": xt, "xtb": xtb, "wqk": wqk, "wvb": wvb, "wob": wob,
            "wosbo": wosbo, "eye": eye, "hm": hm, "mn": mn, "m2": m2,
            "m2o": m2o, "gb": gb,
        })
    return in_maps


def _postprocess(out128):
    return np.ascontiguousarray(
        out128.reshape(N, B, DIM).transpose(1, 0, 2).reshape(B, N, H, W)
    ).astype(np.float32)


def _get_program(reps=1):
    key = ("nc", MM_DT, reps, NO_CC)
    if key not in _PROG_CACHE:
        _PROG_CACHE[key] = _build_program(MM_DT, reps)
    return _PROG_CACHE[key]


def kernel(**inputs):
    from concourse.bass_utils import run_bass_kernel_spmd
    nc = _get_program()
    in_maps = _prep_inputs(**inputs)
    res = run_bass_kernel_spmd(nc, in_maps, list(range(NC)))
    return _postprocess(res.results[0]["out"])


def run_traced(inputs):
    """Like kernel() but with NTFF tracing; returns (output, BassKernelResults)."""
    from concourse.bass_utils import run_bass_kernel_spmd
    nc = _get_program()
    in_maps = _prep_inputs(**inputs)
    res = run_bass_kernel_spmd(nc, in_maps, list(range(NC)), trace=True)
    return _postprocess(res.results[0]["out"]), res


def run_sim(inputs, reps=1):
    """Validate in the multi-core simulator; returns output."""
    from concourse.bass_interp import MultiCoreSim
    nc = _get_program(reps)
    in_maps = _prep_inputs(**inputs)
    sim = MultiCoreSim(nc, num_cores=NC, trace=False)
    for i in range(NC):
        for k, v in in_maps[i].items():
            sim.cores[i].tensor(k)[:] = v
    sim.simulate()
    return _postprocess(np.array(sim.cores[0].tensor("out")))
